# revision 28
# baseline (speedup 1.0000x reference)
"""int4 group-quantized linear: y = x @ dequant(w_packed, w_scale, w_zero).T

Full shapes: x [4096, 4096] f32, W [11008, 4096] int4 (group=128),
y [4096, 11008] f32.

Strategy: column-parallel over 8 NeuronCores. Each core handles 1376
out-features (zero-padded to 1408 = 11*128):
  - host passes x.T (contiguous) so the matmul stationary operand loads
    straight from HBM (no on-chip transposes of x), and w_packed as
    uint8 rows [OPAD, 2048]
  - W prep: nibble extract (xor/mask on DVE + Pool), per-group affine
    (B*s - (8+z)*s) split DVE tensor_scalar / Scalar ACTIVATE-Identity,
    PE-transpose into W.T [i, o] bf16 resident in SBUF
  - main loop: per 512-token chunk DMA x.T g-tiles (SWDGE f32->bf16);
    per 128-token tile accumulate 32 k-groups into PSUM per 512-wide
    output chunk; DVE copies PSUM->SBUF; DMA out. Only the 1376 real
    columns are computed.
"""

import numpy as np

import concourse.bacc as bacc
import concourse.bass as bass
import concourse.mybir as mybir
import concourse.tile as tile
from concourse.bass_utils import run_bass_kernel_spmd
from concourse.masks import make_identity

OUT, IN, TOK, GROUP = 11008, 4096, 4096, 128
NG = IN // GROUP          # 32 groups (= k-tiles)
NCORES = 8
OSH = OUT // NCORES       # 1376 real out-features per core
OTILES = (OSH + 127) // 128   # 11
OPAD = OTILES * 128       # 1408
ROW_BYTES = IN // 2       # 2048 packed bytes per out-feature row
TCHUNK = 512              # tokens per x.T slab
NCHUNK = TOK // TCHUNK    # 8
TPC = TCHUNK // 128       # 4 t-tiles per chunk
OCHUNKS = [(0, 512), (512, 512), (1024, OSH - 1024)]  # 512+512+352

F32 = mybir.dt.float32
BF16 = mybir.dt.bfloat16
I32 = mybir.dt.int32
U8 = mybir.dt.uint8
U16 = mybir.dt.uint16
FP8 = mybir.dt.float8e3
ALU = mybir.AluOpType
ACT = mybir.ActivationFunctionType

# how many of the 32 per-group affine ops go to DVE (rest on Scalar)
AFF_DVE = 18


def build(nc: bass.Bass, variant: str = "base"):
    vs = set(variant.split(","))
    xt_d = nc.dram_tensor("xT", (IN, TOK), F32, kind="ExternalInput")
    wp_d = nc.dram_tensor("wp", (OPAD, ROW_BYTES // 2), U16, kind="ExternalInput")
    ws_d = nc.dram_tensor("ws", (OPAD, NG), F32, kind="ExternalInput")
    wz_d = nc.dram_tensor("wz", (OPAD, NG), I32, kind="ExternalInput")
    y_d = nc.dram_tensor("y", (TOK, OSH), F32, kind="ExternalOutput")

    with tile.TileContext(nc) as tc:
        with tc.tile_pool(name="singles", bufs=1) as singles, \
             tc.tile_pool(name="wtpool", bufs=1) as wtpool, \
             tc.tile_pool(name="prep", bufs=3) as prep, \
             tc.tile_pool(name="prepw", bufs=2) as prepw, \
             tc.tile_pool(name="xpool", bufs=2) as xpool, \
             tc.tile_pool(name="ypool", bufs=4) as ypool, \
             tc.tile_pool(name="ps0", bufs=2, space="PSUM") as ps0, \
             tc.tile_pool(name="ps1", bufs=2, space="PSUM") as ps1, \
             tc.tile_pool(name="ps2", bufs=2, space="PSUM") as ps2, \
             tc.tile_pool(name="psW", bufs=2, space="PSUM") as psW:
            psoc = [ps0, ps1, ps2]

            ident = singles.tile([128, 128], BF16)
            make_identity(nc, ident)

            # W.T resident: [128 i-part, g-major: g*OPAD + o] bf16
            wt = wtpool.tile([128, NG * OPAD], BF16)
            wt_g = wt.rearrange("p (g o) -> p g o", g=NG)

            # ---------------- W prep: dequant + transpose ----------------
            for ot in range(0 if "noprep" in vs else OTILES):
                s_sb = prep.tile([128, NG], F32)
                z_sb = prep.tile([128, NG], I32)
                nc.sync.dma_start(out=s_sb, in_=ws_d.ap()[ot * 128:(ot + 1) * 128, :])
                nc.sync.dma_start(out=z_sb, in_=wz_d.ap()[ot * 128:(ot + 1) * 128, :])
                # nzs8 = -(z + 8) * s  (affine bias)
                z8 = prep.tile([128, NG], F32)
                nc.scalar.activation(out=z8, in_=z_sb, func=ACT.Copy,
                                     scale=-1.0, bias=-8.0)
                nzs8 = prep.tile([128, NG], F32)
                nc.gpsimd.tensor_tensor(out=nzs8, in0=z8, in1=s_sb, op=ALU.mult)

                wp_sb = prep.tile([128, ROW_BYTES // 2], U16)
                nc.sync.dma_start(out=wp_sb,
                                  in_=wp_d.ap()[ot * 128:(ot + 1) * 128, :])

                # nibble extraction in u16 lanes (packed 2-byte ops):
                # t13[:, 0:2048] u8 = B of even i (in order),
                # t13[:, 2048:4096] u8 = B of odd i. B = nibble ^ 8 in [0,15].
                t13 = prep.tile([128, IN], U8)
                t13w = t13.bitcast(U16)
                t2 = prep.tile([128, ROW_BYTES // 2], U16)
                nc.vector.tensor_scalar(
                    out=t13w[:, 0:1024], in0=wp_sb, scalar1=0x8888, scalar2=0x0F0F,
                    op0=ALU.bitwise_xor, op1=ALU.bitwise_and)
                nc.vector.tensor_scalar(
                    out=t2, in0=wp_sb, scalar1=0xF0F0, scalar2=0x8080,
                    op0=ALU.bitwise_and, op1=ALU.bitwise_xor)
                nc.vector.tensor_scalar(
                    out=t13w[:, 1024:2048], in0=t2, scalar1=4, scalar2=None,
                    op0=ALU.logical_shift_right)

                # per-group affine: W = B*s[:,g] - (8+z[:,g])*s[:,g].
                # w_bf column order per g: 64 even i then 64 odd i (the x.T
                # DMA applies the same parity permutation).
                t13h = t13.rearrange("p (h c) -> p h c", h=2)
                w_bf = prepw.tile([128, IN], BF16)
                for g in range(NG):
                    in_ap = t13h[:, :, g * 64:(g + 1) * 64]
                    # natural i order: even nibbles -> cols 0,2,..., odd -> 1,3,...
                    out_ap = w_bf[:, g * 128:(g + 1) * 128].rearrange(
                        "p (c h) -> p h c", h=2)
                    if g % 16 < 6:
                        nc.vector.tensor_scalar(
                            out=out_ap, in0=in_ap,
                            scalar1=s_sb[:, g:g + 1], scalar2=nzs8[:, g:g + 1],
                            op0=ALU.mult, op1=ALU.add)
                    elif g % 16 < 10 and "dmat" not in vs:
                        nc.scalar.activation(
                            out=out_ap, in_=in_ap,
                            func=ACT.Identity,
                            scale=s_sb[:, g:g + 1], bias=nzs8[:, g:g + 1])
                    else:
                        nc.gpsimd.tensor_scalar(
                            out=out_ap, in0=in_ap,
                            scalar1=s_sb[:, g:g + 1], scalar2=nzs8[:, g:g + 1],
                            op0=ALU.mult, op1=ALU.add)

                if "dmat" in vs:
                    # XBAR DMA transpose [o, i] -> [i, o], scalar HWDGE queue
                    for g in range(NG):
                        nc.scalar.dma_start(
                            out=wt_g[:, g, ot * 128:(ot + 1) * 128],
                            in_=w_bf[:, g * 128:(g + 1) * 128],
                            transpose=True)
                else:
                    # PE transpose; 8 groups per PSUM bank (bf16)
                    for gq in range(NG // 8):
                        tpw = psW.tile([128, 1024], BF16)
                        for j in range(8):
                            g = gq * 8 + j
                            nc.tensor.transpose(
                                tpw[:, j * 128:(j + 1) * 128],
                                w_bf[:, g * 128:(g + 1) * 128], ident)
                        out_ap = wt_g[:, gq * 8:(gq + 1) * 8,
                                      ot * 128:(ot + 1) * 128]
                        in_ap = tpw.rearrange("p (j o) -> p j o", j=8)
                        if gq % 2 == 0:
                            nc.vector.tensor_copy(out=out_ap, in_=in_ap)
                        else:
                            nc.scalar.activation(out=out_ap, in_=in_ap,
                                                 func=ACT.Identity)

            if "noprep" in vs:
                nc.gpsimd.memset(wt, 0.001)

            # ---------------- main loop over token chunks ----------------
            for tc_i in range(NCHUNK):
                # x.T slab for this chunk: [128 i-part, g-major 512 t] bf16
                xt = xpool.tile([128, NG * TCHUNK], BF16)
                xt_g = xt.rearrange("p (g t) -> p g t", g=NG)
                # SWDGE cast f32 -> bf16 during DMA. First two chunks load
                # in 8 pieces so the g-loop can start before the whole slab
                # lands; later chunks prefetch far ahead with one cheap
                # descriptor.
                nsub = 8 if tc_i < 2 else 1
                gper = NG // nsub
                for sub in range(nsub):
                    nc.gpsimd.dma_start(
                        out=xt_g[:, sub * gper:(sub + 1) * gper, :],
                        in_=xt_d.ap()[sub * gper * 128:(sub + 1) * gper * 128,
                                      tc_i * TCHUNK:(tc_i + 1) * TCHUNK]
                        .rearrange("(g p) t -> p g t", g=gper))

                if "nomm" in vs:
                    continue

                for tt in range(TPC):
                    t0 = tt * 128
                    for oc, (o0, n) in enumerate(OCHUNKS):
                        yp = psoc[oc].tile([128, 512], F32, name=f"yp{oc}",
                                           tag=f"yp{oc}")
                        for g in range(NG):
                            nc.tensor.matmul(
                                yp[:, :n],
                                xt_g[:, g, t0:t0 + 128],
                                wt[:, g * OPAD + o0: g * OPAD + o0 + n],
                                start=(g == 0), stop=(g == NG - 1))
                        y_sb = ypool.tile([128, 512], F32, name="ysb", tag="ysb")
                        nc.vector.tensor_copy(out=y_sb[:, :n], in_=yp[:, :n])
                        nc.sync.dma_start(
                            out=y_d.ap()[tc_i * TCHUNK + t0:
                                         tc_i * TCHUNK + t0 + 128, o0:o0 + n],
                            in_=y_sb[:, :n])


_nc_cache = None


def _get_nc():
    global _nc_cache
    if _nc_cache is None:
        import os
        nc = bacc.Bacc("TRN2", target_bir_lowering=False, debug=False)
        build(nc, variant=os.environ.get("BASS_VARIANT", "base"))
        nc.compile()
        _nc_cache = nc
    return _nc_cache


def make_in_maps(x, w_packed, w_scale, w_zero):
    x = np.asarray(x, dtype=np.float32)
    xT = np.ascontiguousarray(x.T)                      # [IN, TOK]
    wp = np.asarray(w_packed, dtype=np.int32).astype(np.uint8)
    wp = wp.reshape(OUT, ROW_BYTES).view(np.uint16)     # [OUT, 1024] LE pairs
    ws = np.asarray(w_scale, dtype=np.float32)
    wz = np.asarray(w_zero, dtype=np.int32)

    in_maps = []
    for c in range(NCORES):
        sl = slice(c * OSH, (c + 1) * OSH)
        wp_c = np.zeros((OPAD, ROW_BYTES // 2), dtype=np.uint16)
        wp_c[:OSH] = wp[sl]
        ws_c = np.zeros((OPAD, NG), dtype=np.float32)
        ws_c[:OSH] = ws[sl]
        wz_c = np.zeros((OPAD, NG), dtype=np.int32)
        wz_c[:OSH] = wz[sl]
        in_maps.append({"xT": xT, "wp": wp_c, "ws": ws_c, "wz": wz_c})
    return in_maps


def kernel(x, w_packed, w_scale, w_zero):
    nc = _get_nc()
    in_maps = make_in_maps(x, w_packed, w_scale, w_zero)
    res = run_bass_kernel_spmd(nc, in_maps, core_ids=list(range(NCORES)))
    y = np.concatenate([res.results[c]["y"] for c in range(NCORES)], axis=1)
    return y.astype(np.float32)


# revision 29
# speedup vs baseline: 1.0060x; 1.0060x over previous
"""int4 group-quantized linear: y = x @ dequant(w_packed, w_scale, w_zero).T

Full shapes: x [4096, 4096] f32, W [11008, 4096] int4 (group=128),
y [4096, 11008] f32.

Strategy: column-parallel over 8 NeuronCores. Each core handles 1376
out-features (zero-padded to 1408 = 11*128):
  - host passes x.T (contiguous) so the matmul stationary operand loads
    straight from HBM (no on-chip transposes of x), and w_packed as
    uint8 rows [OPAD, 2048]
  - W prep: nibble extract (xor/mask on DVE + Pool), per-group affine
    (B*s - (8+z)*s) split DVE tensor_scalar / Scalar ACTIVATE-Identity,
    PE-transpose into W.T [i, o] bf16 resident in SBUF
  - main loop: per 512-token chunk DMA x.T g-tiles (SWDGE f32->bf16);
    per 128-token tile accumulate 32 k-groups into PSUM per 512-wide
    output chunk; DVE copies PSUM->SBUF; DMA out. Only the 1376 real
    columns are computed.
"""

import numpy as np

import concourse.bacc as bacc
import concourse.bass as bass
import concourse.mybir as mybir
import concourse.tile as tile
from concourse.bass_utils import run_bass_kernel_spmd
from concourse.masks import make_identity

OUT, IN, TOK, GROUP = 11008, 4096, 4096, 128
NG = IN // GROUP          # 32 groups (= k-tiles)
NCORES = 8
OSH = OUT // NCORES       # 1376 real out-features per core
OTILES = (OSH + 127) // 128   # 11
OPAD = OTILES * 128       # 1408
ROW_BYTES = IN // 2       # 2048 packed bytes per out-feature row
TCHUNK = 512              # tokens per x.T slab
NCHUNK = TOK // TCHUNK    # 8
TPC = TCHUNK // 128       # 4 t-tiles per chunk
OCHUNKS = [(0, 512), (512, 512), (1024, OSH - 1024)]  # 512+512+352

F32 = mybir.dt.float32
BF16 = mybir.dt.bfloat16
I32 = mybir.dt.int32
U8 = mybir.dt.uint8
U16 = mybir.dt.uint16
FP8 = mybir.dt.float8e3
ALU = mybir.AluOpType
ACT = mybir.ActivationFunctionType

# how many of the 32 per-group affine ops go to DVE (rest on Scalar)
AFF_DVE = 18


def build(nc: bass.Bass, variant: str = "base"):
    vs = set(variant.split(","))
    xt_d = nc.dram_tensor("xT", (IN, TOK), F32, kind="ExternalInput")
    wp_d = nc.dram_tensor("wp", (OPAD, ROW_BYTES // 2), U16, kind="ExternalInput")
    ws_d = nc.dram_tensor("ws", (OPAD, NG), F32, kind="ExternalInput")
    wz_d = nc.dram_tensor("wz", (OPAD, NG), I32, kind="ExternalInput")
    y_d = nc.dram_tensor("y", (TOK, OSH), F32, kind="ExternalOutput")

    with tile.TileContext(nc) as tc:
        with tc.tile_pool(name="singles", bufs=1) as singles, \
             tc.tile_pool(name="wtpool", bufs=1) as wtpool, \
             tc.tile_pool(name="prep", bufs=3) as prep, \
             tc.tile_pool(name="prepw", bufs=2) as prepw, \
             tc.tile_pool(name="xpool", bufs=2) as xpool, \
             tc.tile_pool(name="ypool", bufs=4) as ypool, \
             tc.tile_pool(name="ps0", bufs=2, space="PSUM") as ps0, \
             tc.tile_pool(name="ps1", bufs=2, space="PSUM") as ps1, \
             tc.tile_pool(name="ps2", bufs=2, space="PSUM") as ps2, \
             tc.tile_pool(name="psW", bufs=2, space="PSUM") as psW:
            psoc = [ps0, ps1, ps2]

            ident = singles.tile([128, 128], BF16)
            make_identity(nc, ident)

            # W.T resident: [128 i-part, g-major: g*OPAD + o] bf16
            wt = wtpool.tile([128, NG * OPAD], BF16)
            wt_g = wt.rearrange("p (g o) -> p g o", g=NG)

            # ---------------- W prep: dequant + transpose ----------------
            for ot in range(0 if "noprep" in vs else OTILES):
                s_sb = prep.tile([128, NG], F32)
                z_sb = prep.tile([128, NG], I32)
                nc.sync.dma_start(out=s_sb, in_=ws_d.ap()[ot * 128:(ot + 1) * 128, :])
                nc.sync.dma_start(out=z_sb, in_=wz_d.ap()[ot * 128:(ot + 1) * 128, :])
                # nzs8 = -(z + 8) * s  (affine bias)
                z8 = prep.tile([128, NG], F32)
                nc.scalar.activation(out=z8, in_=z_sb, func=ACT.Copy,
                                     scale=-1.0, bias=-8.0)
                nzs8 = prep.tile([128, NG], F32)
                nc.gpsimd.tensor_tensor(out=nzs8, in0=z8, in1=s_sb, op=ALU.mult)

                wp_sb = prep.tile([128, ROW_BYTES // 2], U16)
                nc.sync.dma_start(out=wp_sb,
                                  in_=wp_d.ap()[ot * 128:(ot + 1) * 128, :])

                # nibble extraction in u16 lanes (packed 2-byte ops):
                # t13[:, 0:2048] u8 = B of even i (in order),
                # t13[:, 2048:4096] u8 = B of odd i. B = nibble ^ 8 in [0,15].
                t13 = prep.tile([128, IN], U8)
                t13w = t13.bitcast(U16)
                t2 = prep.tile([128, ROW_BYTES // 2], U16)
                nc.vector.tensor_scalar(
                    out=t13w[:, 0:1024], in0=wp_sb, scalar1=0x8888, scalar2=0x0F0F,
                    op0=ALU.bitwise_xor, op1=ALU.bitwise_and)
                nc.vector.tensor_scalar(
                    out=t2, in0=wp_sb, scalar1=0xF0F0, scalar2=0x8080,
                    op0=ALU.bitwise_and, op1=ALU.bitwise_xor)
                nc.vector.tensor_scalar(
                    out=t13w[:, 1024:2048], in0=t2, scalar1=4, scalar2=None,
                    op0=ALU.logical_shift_right)

                # per-group affine: W = B*s[:,g] - (8+z[:,g])*s[:,g].
                # w_bf column order per g: 64 even i then 64 odd i (the x.T
                # DMA applies the same parity permutation).
                t13h = t13.rearrange("p (h c) -> p h c", h=2)
                w_bf = prepw.tile([128, IN], BF16)
                for g in range(NG):
                    in_ap = t13h[:, :, g * 64:(g + 1) * 64]
                    # natural i order: even nibbles -> cols 0,2,..., odd -> 1,3,...
                    out_ap = w_bf[:, g * 128:(g + 1) * 128].rearrange(
                        "p (c h) -> p h c", h=2)
                    if g % 16 < 6:
                        nc.vector.tensor_scalar(
                            out=out_ap, in0=in_ap,
                            scalar1=s_sb[:, g:g + 1], scalar2=nzs8[:, g:g + 1],
                            op0=ALU.mult, op1=ALU.add)
                    elif g % 16 < 11 and "dmat" not in vs:
                        nc.scalar.activation(
                            out=out_ap, in_=in_ap,
                            func=ACT.Identity,
                            scale=s_sb[:, g:g + 1], bias=nzs8[:, g:g + 1])
                    else:
                        nc.gpsimd.tensor_scalar(
                            out=out_ap, in0=in_ap,
                            scalar1=s_sb[:, g:g + 1], scalar2=nzs8[:, g:g + 1],
                            op0=ALU.mult, op1=ALU.add)

                if "dmat" in vs:
                    # XBAR DMA transpose [o, i] -> [i, o], scalar HWDGE queue
                    for g in range(NG):
                        nc.scalar.dma_start(
                            out=wt_g[:, g, ot * 128:(ot + 1) * 128],
                            in_=w_bf[:, g * 128:(g + 1) * 128],
                            transpose=True)
                else:
                    # PE transpose; 8 groups per PSUM bank (bf16)
                    for gq in range(NG // 8):
                        tpw = psW.tile([128, 1024], BF16)
                        for j in range(8):
                            g = gq * 8 + j
                            nc.tensor.transpose(
                                tpw[:, j * 128:(j + 1) * 128],
                                w_bf[:, g * 128:(g + 1) * 128], ident)
                        out_ap = wt_g[:, gq * 8:(gq + 1) * 8,
                                      ot * 128:(ot + 1) * 128]
                        in_ap = tpw.rearrange("p (j o) -> p j o", j=8)
                        if gq % 2 == 0:
                            nc.vector.tensor_copy(out=out_ap, in_=in_ap)
                        else:
                            nc.scalar.activation(out=out_ap, in_=in_ap,
                                                 func=ACT.Identity)

            if "noprep" in vs:
                nc.gpsimd.memset(wt, 0.001)

            # ---------------- main loop over token chunks ----------------
            for tc_i in range(NCHUNK):
                # x.T slab for this chunk: [128 i-part, g-major 512 t] bf16
                xt = xpool.tile([128, NG * TCHUNK], BF16)
                xt_g = xt.rearrange("p (g t) -> p g t", g=NG)
                # SWDGE cast f32 -> bf16 during DMA. First two chunks load
                # in 8 pieces so the g-loop can start before the whole slab
                # lands; later chunks prefetch far ahead with one cheap
                # descriptor.
                nsub = 4 if tc_i < 2 else 1
                gper = NG // nsub
                for sub in range(nsub):
                    nc.gpsimd.dma_start(
                        out=xt_g[:, sub * gper:(sub + 1) * gper, :],
                        in_=xt_d.ap()[sub * gper * 128:(sub + 1) * gper * 128,
                                      tc_i * TCHUNK:(tc_i + 1) * TCHUNK]
                        .rearrange("(g p) t -> p g t", g=gper))

                if "nomm" in vs:
                    continue

                for tt in range(TPC):
                    t0 = tt * 128
                    for oc, (o0, n) in enumerate(OCHUNKS):
                        yp = psoc[oc].tile([128, 512], F32, name=f"yp{oc}",
                                           tag=f"yp{oc}")
                        for g in range(NG):
                            nc.tensor.matmul(
                                yp[:, :n],
                                xt_g[:, g, t0:t0 + 128],
                                wt[:, g * OPAD + o0: g * OPAD + o0 + n],
                                start=(g == 0), stop=(g == NG - 1))
                        y_sb = ypool.tile([128, 512], F32, name="ysb", tag="ysb")
                        nc.vector.tensor_copy(out=y_sb[:, :n], in_=yp[:, :n])
                        nc.sync.dma_start(
                            out=y_d.ap()[tc_i * TCHUNK + t0:
                                         tc_i * TCHUNK + t0 + 128, o0:o0 + n],
                            in_=y_sb[:, :n])


_nc_cache = None


def _get_nc():
    global _nc_cache
    if _nc_cache is None:
        import os
        nc = bacc.Bacc("TRN2", target_bir_lowering=False, debug=False)
        build(nc, variant=os.environ.get("BASS_VARIANT", "base"))
        nc.compile()
        _nc_cache = nc
    return _nc_cache


def make_in_maps(x, w_packed, w_scale, w_zero):
    x = np.asarray(x, dtype=np.float32)
    xT = np.ascontiguousarray(x.T)                      # [IN, TOK]
    wp = np.asarray(w_packed, dtype=np.int32).astype(np.uint8)
    wp = wp.reshape(OUT, ROW_BYTES).view(np.uint16)     # [OUT, 1024] LE pairs
    ws = np.asarray(w_scale, dtype=np.float32)
    wz = np.asarray(w_zero, dtype=np.int32)

    in_maps = []
    for c in range(NCORES):
        sl = slice(c * OSH, (c + 1) * OSH)
        wp_c = np.zeros((OPAD, ROW_BYTES // 2), dtype=np.uint16)
        wp_c[:OSH] = wp[sl]
        ws_c = np.zeros((OPAD, NG), dtype=np.float32)
        ws_c[:OSH] = ws[sl]
        wz_c = np.zeros((OPAD, NG), dtype=np.int32)
        wz_c[:OSH] = wz[sl]
        in_maps.append({"xT": xT, "wp": wp_c, "ws": ws_c, "wz": wz_c})
    return in_maps


def kernel(x, w_packed, w_scale, w_zero):
    nc = _get_nc()
    in_maps = make_in_maps(x, w_packed, w_scale, w_zero)
    res = run_bass_kernel_spmd(nc, in_maps, core_ids=list(range(NCORES)))
    y = np.concatenate([res.results[c]["y"] for c in range(NCORES)], axis=1)
    return y.astype(np.float32)
